# revision 11
# baseline (speedup 1.0000x reference)
"""Trainium2 Bass kernel for a multi-head attention block (B=4, T=2048, H=16, D=64).

Computation (matches the reference):
    q = LN(inputs_q @ Wq.T) ; k = LN(inputs_kv @ Wk.T) ; v = inputs_kv @ Wv.T
    out = softmax(q k^T / sqrt(D)) @ v ; y = out @ Wo.T

Sharding: 8 shards = (batch b, half of T).  Core c handles the 1024 query
tokens (b = c//2, half = c%2) and computes K/V for the full 2048 tokens of
batch b (K/V projection duplicated within each core pair; avoids
collectives).  Each core is self-contained; the host only slices /
transposes / concatenates.

Per-core pipeline:
  - fp32r (full-speed fp32) matmuls for the Q/K/V projections,
    LayerNorm via bn_stats/bn_aggr on VectorE, rstd = Exp(-0.5*Ln(var+eps))
    so the only ACT table set used anywhere is ln/exp.
  - LN output is PE-transposed to [E, tok] layout (bf16), with gamma/beta
    applied per-partition during the PSUM->SBUF copy.
  - Attention per head with *transposed* scores sT[tok_k, tok_q] =
    k_h q_h^T, exp on ScalarE straight out of PSUM (scale=1/8 folded in),
    and o^T = [v_h | 1]^T @ exp(sT) so the softmax denominator falls out of
    the same accumulation (row 64).  Normalization multiplies by
    exp(-Ln(sum)) broadcast across partitions via a rank-1 matmul.
  - Output projection in fp32r from the already-transposed o^T.
"""

import os
import sys
from contextlib import ExitStack

sys.path.insert(0, "/opt/trn_rl_repo")

import numpy as np

B, T, H, D = 4, 2048, 16, 64
E = H * D  # 1024
EPS = 1e-5
NCORES = 8
TQ = T // 2   # query tokens per core
TK = T        # kv tokens per core
KT = E // 128  # 8 contraction tiles
VW = D + 1     # v columns per head incl. the ones column

_CACHE = {}


def _patch_act_tables():
    # The kernel uses only Ln and Exp. Left alone, bass places Exp in
    # "exp_and_others" and Ln in "natural_log_exp_and_others" and emits a
    # ~1.3us ACT table reload on every switch (80 reloads, ~100us).  Hide
    # Exp from every set that lacks Ln so both functions resolve to the one
    # set that holds them together.  Set count/order (and thus
    # act_func_set_id indices) are unchanged.
    import concourse.hw_specs as hw_specs
    from concourse import bacc, mybir

    if getattr(_patch_act_tables, "_done", False):
        return
    _patch_act_tables._done = True
    orig = hw_specs.get_activation_tables
    Exp = mybir.ActivationFunctionType.Exp
    Ln = mybir.ActivationFunctionType.Ln

    def patched(arch):
        out = {}
        for name, funcs in orig(arch).items():
            if Exp in funcs and Ln not in funcs:
                funcs = funcs - {Exp}
            out[name] = funcs
        return out

    hw_specs.get_activation_tables = patched
    bacc.get_activation_tables = patched


def _build():
    import concourse.tile as tile
    from concourse import bacc, mybir
    from concourse.masks import make_identity

    _patch_act_tables()

    f32 = mybir.dt.float32
    f32r = mybir.dt.float32r
    bf16 = mybir.dt.bfloat16
    Ln = mybir.ActivationFunctionType.Ln
    Exp = mybir.ActivationFunctionType.Exp
    SUB = mybir.AluOpType.subtract
    MULT = mybir.AluOpType.mult
    ADD = mybir.AluOpType.add

    nc = bacc.Bacc("TRN2", target_bir_lowering=False, debug=False,
                   num_devices=NCORES)

    xqT = nc.dram_tensor("xqT", [128, KT, TQ], f32r, kind="ExternalInput").ap()
    xkvT = nc.dram_tensor("xkvT", [128, KT, TK], f32r, kind="ExternalInput").ap()
    wqT = nc.dram_tensor("wqT", [128, KT, E], f32r, kind="ExternalInput").ap()
    wkT = nc.dram_tensor("wkT", [128, KT, E], f32r, kind="ExternalInput").ap()
    wvT = nc.dram_tensor("wvT", [128, KT, E], f32r, kind="ExternalInput").ap()
    woT = nc.dram_tensor("woT", [128, KT, E], f32r, kind="ExternalInput").ap()
    gq = nc.dram_tensor("gq", [128, KT], f32, kind="ExternalInput").ap()
    bq = nc.dram_tensor("bq", [128, KT], f32, kind="ExternalInput").ap()
    gk = nc.dram_tensor("gk", [128, KT], f32, kind="ExternalInput").ap()
    bk = nc.dram_tensor("bk", [128, KT], f32, kind="ExternalInput").ap()
    y = nc.dram_tensor("y", [TQ, E], f32, kind="ExternalOutput").ap()

    with tile.TileContext(nc) as tc, ExitStack() as top:
        const = top.enter_context(tc.tile_pool(name="const", bufs=1))
        ident = const.tile([128, 128], bf16)
        make_identity(nc, ident[:])
        ones64 = const.tile([1, 64], f32)
        nc.vector.memset(ones64[:], 1.0)
        eps_sb = const.tile([128, 1], f32)
        nc.vector.memset(eps_sb[:], EPS)
        zero_sb = const.tile([128, 1], f32)
        nc.vector.memset(zero_sb[:], 0.0)
        gq_sb = const.tile([128, KT], f32)
        nc.sync.dma_start(out=gq_sb[:], in_=gq)
        bq_sb = const.tile([128, KT], f32)
        nc.sync.dma_start(out=bq_sb[:], in_=bq)
        gk_sb = const.tile([128, KT], f32)
        nc.sync.dma_start(out=gk_sb[:], in_=gk)
        bk_sb = const.tile([128, KT], f32)
        nc.sync.dma_start(out=bk_sb[:], in_=bk)

        pers = top.enter_context(tc.tile_pool(name="pers", bufs=1))
        qT = pers.tile([128, KT, TQ], bf16)       # LN(q)^T, [E, tok_q]
        kTt = pers.tile([128, KT, TK], bf16)      # LN(k)^T, [E, tok_k]
        vsb = pers.tile([128, TK // 128, H * VW], bf16)  # v + ones col per head
        oT = pers.tile([128, KT, TQ], f32r)        # attention output^T, [E, tok_q]
        nc.vector.memset(
            vsb[:].rearrange("p k (h w) -> p k h w", w=VW)[:, :, :, 64:65], 1.0)

        def project(ps, x_t, w_sb):
            # ps[tok, E] += x^T.T @ W^T  (fp32r, full PE rate)
            for nt in range(2):
                for kt in range(KT):
                    nc.tensor.matmul(
                        ps[:, nt * 512:(nt + 1) * 512],
                        lhsT=x_t[:, kt, :],
                        rhs=w_sb[:, kt, nt * 512:(nt + 1) * 512],
                        start=(kt == 0), stop=(kt == KT - 1))

        def ln_transpose(pools, ps, gam, bet, dstT, mt):
            stats, qn_pool, psT = pools
            st = stats.tile([128, 2, 6], f32, tag="st")
            nc.vector.bn_stats(st[:, 0, :], ps[:, 0:512])
            nc.vector.bn_stats(st[:, 1, :], ps[:, 512:1024])
            mv = stats.tile([128, 2], f32, tag="mv")
            nc.vector.bn_aggr(mv[:], st[:])
            lnv = stats.tile([128, 1], f32, tag="lnv")
            nc.scalar.activation(lnv[:], mv[:, 1:2], Ln, bias=eps_sb[:])
            rstd = stats.tile([128, 1], f32, tag="rstd")
            nc.scalar.activation(rstd[:], lnv[:], Exp, bias=zero_sb[:], scale=-0.5)
            qn = qn_pool.tile([128, E], bf16, tag="qn")
            nc.vector.tensor_scalar(out=qn[:], in0=ps[:], scalar1=mv[:, 0:1],
                                    scalar2=rstd[:], op0=SUB, op1=MULT)
            for kt in range(KT):
                tp = psT.tile([128, 128], bf16, tag="tp")
                nc.tensor.transpose(tp[:], qn[:, kt * 128:(kt + 1) * 128], ident[:])
                nc.vector.tensor_scalar(
                    out=dstT[:, kt, mt * 128:(mt + 1) * 128], in0=tp[:],
                    scalar1=gam[:, kt:kt + 1], scalar2=bet[:, kt:kt + 1],
                    op0=MULT, op1=ADD)

        # ---- Phase A: projections + LN + transposes ----
        with ExitStack() as phA:
            wpool = phA.enter_context(tc.tile_pool(name="wA", bufs=2))
            xpool = phA.enter_context(tc.tile_pool(name="xA", bufs=2))
            psA = phA.enter_context(tc.tile_pool(name="psA", bufs=2, space="PSUM"))
            psT = phA.enter_context(tc.tile_pool(name="psT", bufs=4, space="PSUM"))
            stats = phA.enter_context(tc.tile_pool(name="stats", bufs=3))
            qn_pool = phA.enter_context(tc.tile_pool(name="qn", bufs=3))
            pools = (stats, qn_pool, psT)

            # Wq and Wv share a slot: Wq is dead after the Q loop, before the
            # K/V loop needs Wv (the Wv DMA waits for Wq's last matmul).
            wq_sb = wpool.tile([128, KT, E], f32r, tag="w")
            nc.sync.dma_start(out=wq_sb[:], in_=wqT)
            wk_sb = wpool.tile([128, KT, E], f32r, tag="w")
            nc.sync.dma_start(out=wk_sb[:], in_=wkT)

            for mt in range(TQ // 128):  # Q projection, 1024 query tokens
                x_t = xpool.tile([128, KT, 128], f32r, tag="xt")
                nc.sync.dma_start(out=x_t[:], in_=xqT[:, :, mt * 128:(mt + 1) * 128])
                ps = psA.tile([128, E], f32, tag="psA")
                project(ps, x_t, wq_sb)
                ln_transpose(pools, ps, gq_sb, bq_sb, qT, mt)

            wv_sb = wpool.tile([128, KT, E], f32r, tag="w")
            nc.sync.dma_start(out=wv_sb[:], in_=wvT)
            for mt in range(TK // 128):  # K + V projections, 2048 kv tokens
                x_t = xpool.tile([128, KT, 128], f32r, tag="xt")
                nc.sync.dma_start(out=x_t[:], in_=xkvT[:, :, mt * 128:(mt + 1) * 128])
                psk = psA.tile([128, E], f32, tag="psA")
                project(psk, x_t, wk_sb)
                psv = psA.tile([128, E], f32, tag="psA")
                project(psv, x_t, wv_sb)
                ln_transpose(pools, psk, gk_sb, bk_sb, kTt, mt)
                for nt in range(2):
                    nc.vector.tensor_copy(
                        out=vsb[:, mt, :].rearrange("p (h w) -> p h w", w=VW)
                            [:, nt * 8:(nt + 1) * 8, 0:64],
                        in_=psv[:, nt * 512:(nt + 1) * 512]
                            .rearrange("p (h w) -> p h w", w=64))

        # ---- Phase B: attention (transposed scores) + output projection ----
        with ExitStack() as phB:
            wpoolB = phB.enter_context(tc.tile_pool(name="wB", bufs=1))
            wo_sb = wpoolB.tile([128, KT, E], f32r)
            nc.sync.dma_start(out=wo_sb[:], in_=woT)

            attn = phB.enter_context(ExitStack())
            sT_ps = attn.enter_context(tc.tile_pool(name="sT", bufs=2, space="PSUM"))
            o_ps = attn.enter_context(tc.tile_pool(name="ops", bufs=2, space="PSUM"))
            ex_pool = attn.enter_context(tc.tile_pool(name="ex", bufs=3))
            oraw_pool = attn.enter_context(tc.tile_pool(name="oraw", bufs=2))
            rows = attn.enter_context(tc.tile_pool(name="rows", bufs=3))

            for h in range(H):
                et, pr = h // 2, (h % 2) * 64
                o = o_ps.tile([65, TQ], f32, tag="ops")
                for kt in range(TK // 128):
                    sps = sT_ps.tile([128, TQ], f32, tag="sT")
                    for nt in range(2):
                        nc.tensor.matmul(
                            sps[:, nt * 512:(nt + 1) * 512],
                            lhsT=kTt[pr:pr + 64, et, kt * 128:(kt + 1) * 128],
                            rhs=qT[pr:pr + 64, et, nt * 512:(nt + 1) * 512])
                    ex = ex_pool.tile([128, TQ], bf16, tag="ex")
                    nc.scalar.activation(ex[:], sps[:], Exp, bias=zero_sb[:], scale=0.125)
                    for nt in range(2):
                        nc.tensor.matmul(
                            o[:, nt * 512:(nt + 1) * 512],
                            lhsT=vsb[:, kt, h * VW:(h + 1) * VW],
                            rhs=ex[:, nt * 512:(nt + 1) * 512],
                            start=(kt == 0), stop=(kt == TK // 128 - 1))
                oraw = oraw_pool.tile([65, TQ], f32, tag="oraw")
                nc.vector.tensor_copy(out=oraw[:], in_=o[:])
                logr = rows.tile([1, TQ], f32, tag="rows")
                nc.scalar.activation(logr[:], oraw[64:65, :], Ln, bias=zero_sb[0:1, :])
                recr = rows.tile([1, TQ], f32, tag="rows")
                nc.scalar.activation(recr[:], logr[:], Exp, bias=zero_sb[0:1, :], scale=-1.0)
                rep = o_ps.tile([64, TQ], f32, tag="ops")
                for nt in range(2):
                    nc.tensor.matmul(
                        rep[:, nt * 512:(nt + 1) * 512],
                        lhsT=ones64[:],
                        rhs=recr[0:1, nt * 512:(nt + 1) * 512])
                nc.vector.tensor_mul(out=oT[pr:pr + 64, et, :],
                                     in0=oraw[0:64, :], in1=rep[:])

            attn.close()
            psY = phB.enter_context(tc.tile_pool(name="psY", bufs=2, space="PSUM"))
            ypool = phB.enter_context(tc.tile_pool(name="yp", bufs=2))
            for mt in range(TQ // 128):
                yps = psY.tile([128, E], f32, tag="psY")
                for nt in range(2):
                    for kt in range(KT):
                        nc.tensor.matmul(
                            yps[:, nt * 512:(nt + 1) * 512],
                            lhsT=oT[:, kt, mt * 128:(mt + 1) * 128],
                            rhs=wo_sb[:, kt, nt * 512:(nt + 1) * 512],
                            start=(kt == 0), stop=(kt == KT - 1))
                ysb = ypool.tile([128, E], f32, tag="ysb")
                nc.vector.tensor_copy(out=ysb[:], in_=yps[:])
                nc.sync.dma_start(out=y[mt * 128:(mt + 1) * 128, :], in_=ysb[:])

    nc.compile()
    return nc


def _tile_t(a):
    # [tok, E] -> [128, KT, tok] with [p, kt, t] = a[t, kt*128+p]
    return np.ascontiguousarray(a.T.reshape(KT, 128, -1).transpose(1, 0, 2))


def _shard(inputs):
    wq = _tile_t(np.ascontiguousarray(inputs["Wq"]))   # Wq.T tiled
    wk = _tile_t(np.ascontiguousarray(inputs["Wk"]))
    wv = _tile_t(np.ascontiguousarray(inputs["Wv"]))
    wo = _tile_t(np.ascontiguousarray(inputs["Wo"]))
    gq = np.ascontiguousarray(inputs["q_ln_gamma"].reshape(KT, 128).T)
    bq = np.ascontiguousarray(inputs["q_ln_beta"].reshape(KT, 128).T)
    gk = np.ascontiguousarray(inputs["k_ln_gamma"].reshape(KT, 128).T)
    bk = np.ascontiguousarray(inputs["k_ln_beta"].reshape(KT, 128).T)
    in_maps = []
    for c in range(NCORES):
        b, half = c // 2, c % 2
        xq = inputs["inputs_q"][b, half * TQ:(half + 1) * TQ, :]
        xkv = inputs["inputs_kv"][b]
        in_maps.append({
            "xqT": _tile_t(xq), "xkvT": _tile_t(xkv),
            "wqT": wq, "wkT": wk, "wvT": wv, "woT": wo,
            "gq": gq, "bq": bq, "gk": gk, "bk": bk,
        })
    return in_maps


def run_sharded(inputs, trace=False):
    from concourse.bass_utils import run_bass_kernel_spmd
    if "nc" not in _CACHE:
        _CACHE["nc"] = _build()
    nc = _CACHE["nc"]
    in_maps = _shard(inputs)
    res = run_bass_kernel_spmd(nc, in_maps, core_ids=list(range(NCORES)),
                               trace=trace)
    out = np.empty((B, T, E), np.float32)
    for c in range(NCORES):
        b, half = c // 2, c % 2
        out[b, half * TQ:(half + 1) * TQ, :] = res.results[c]["y"]
    return out, res


def kernel(**inputs):
    inputs = {k: np.asarray(v, np.float32) for k, v in inputs.items()}
    out, _ = run_sharded(inputs,
                         trace=bool(int(os.environ.get("KERNEL_TRACE", "0"))))
    return out


# revision 25
# speedup vs baseline: 78.3507x; 78.3507x over previous
"""Trainium2 Bass kernel for a multi-head attention block (B=4, T=2048, H=16, D=64).

Computation (matches the reference):
    q = LN(inputs_q @ Wq.T) ; k = LN(inputs_kv @ Wk.T) ; v = inputs_kv @ Wv.T
    out = softmax(q k^T / sqrt(D)) @ v ; y = out @ Wo.T

Sharding: 8 shards = (batch b, half of T).  Core c handles the 1024 query
tokens (b = c//2, half = c%2) and computes K/V for the full 2048 tokens of
batch b (K/V projection duplicated within each core pair; avoids
collectives).  Each core is self-contained; the host only slices /
transposes / concatenates.

Per-core pipeline:
  - fp32r (full-speed fp32) matmuls for the Q/K/V projections,
    LayerNorm via bn_stats/bn_aggr on VectorE, rstd = Exp(-0.5*Ln(var+eps))
    so the only ACT table set used anywhere is ln/exp.
  - LN output is PE-transposed to [E, tok] layout (bf16), with gamma/beta
    applied per-partition during the PSUM->SBUF copy.
  - Attention per head with *transposed* scores sT[tok_k, tok_q] =
    k_h q_h^T, exp on ScalarE straight out of PSUM (scale=1/8 folded in),
    and o^T = [v_h | 1]^T @ exp(sT) so the softmax denominator falls out of
    the same accumulation (row 64).  Normalization multiplies by
    exp(-Ln(sum)); the row is broadcast across the 64 head partitions by a
    DRAM round-trip DMA with a 0-stride partition AP, and each head's
    normalize is deferred by one head so ScalarE's FIFO never stalls.
  - Output projection in fp32r from the already-transposed o^T.
"""

import os
import sys
from contextlib import ExitStack

sys.path.insert(0, "/opt/trn_rl_repo")

import numpy as np

B, T, H, D = 4, 2048, 16, 64
E = H * D  # 1024
EPS = 1e-5
NCORES = 8
TQ = T // 2   # query tokens per core
TK = T        # kv tokens per core
KT = E // 128  # 8 contraction tiles
VW = D + 1     # v columns per head incl. the ones column

_CACHE = {}


def _patch_act_tables():
    # The kernel uses only Ln and Exp. Left alone, bass places Exp in
    # "exp_and_others" and Ln in "natural_log_exp_and_others" and emits a
    # ~1.3us ACT table reload on every switch (80 reloads, ~100us).  Hide
    # Exp from every set that lacks Ln so both functions resolve to the one
    # set that holds them together.  Set count/order (and thus
    # act_func_set_id indices) are unchanged.
    import concourse.hw_specs as hw_specs
    from concourse import bacc, mybir

    if getattr(_patch_act_tables, "_done", False):
        return
    _patch_act_tables._done = True
    orig = hw_specs.get_activation_tables
    Exp = mybir.ActivationFunctionType.Exp
    Ln = mybir.ActivationFunctionType.Ln

    def patched(arch):
        out = {}
        for name, funcs in orig(arch).items():
            if Exp in funcs and Ln not in funcs:
                funcs = funcs - {Exp}
            out[name] = funcs
        return out

    hw_specs.get_activation_tables = patched
    bacc.get_activation_tables = patched


def _build():
    import concourse.tile as tile
    from concourse import bacc, mybir
    from concourse.masks import make_identity

    _patch_act_tables()

    f32 = mybir.dt.float32
    f32r = mybir.dt.float32r
    bf16 = mybir.dt.bfloat16
    Ln = mybir.ActivationFunctionType.Ln
    Exp = mybir.ActivationFunctionType.Exp
    SUB = mybir.AluOpType.subtract
    MULT = mybir.AluOpType.mult
    ADD = mybir.AluOpType.add

    nc = bacc.Bacc("TRN2", target_bir_lowering=False, debug=False,
                   num_devices=NCORES)

    xqT = nc.dram_tensor("xqT", [128, KT, TQ], f32r, kind="ExternalInput").ap()
    xkvT = nc.dram_tensor("xkvT", [128, KT, TK], f32r, kind="ExternalInput").ap()
    wqT = nc.dram_tensor("wqT", [128, KT, E], f32r, kind="ExternalInput").ap()
    wkT = nc.dram_tensor("wkT", [128, KT, E], f32r, kind="ExternalInput").ap()
    wvT = nc.dram_tensor("wvT", [128, KT, E], f32r, kind="ExternalInput").ap()
    woT = nc.dram_tensor("woT", [128, KT, E], f32r, kind="ExternalInput").ap()
    gq = nc.dram_tensor("gq", [128, KT], f32, kind="ExternalInput").ap()
    bq = nc.dram_tensor("bq", [128, KT], f32, kind="ExternalInput").ap()
    gk = nc.dram_tensor("gk", [128, KT], f32, kind="ExternalInput").ap()
    bk = nc.dram_tensor("bk", [128, KT], f32, kind="ExternalInput").ap()
    y = nc.dram_tensor("y", [TQ, E], f32, kind="ExternalOutput").ap()

    with tile.TileContext(nc) as tc, ExitStack() as top:
        const = top.enter_context(tc.tile_pool(name="const", bufs=1))
        ident = const.tile([128, 128], bf16)
        make_identity(nc, ident[:])
        eps_sb = const.tile([128, 1], f32)
        nc.vector.memset(eps_sb[:], EPS)
        zero_sb = const.tile([128, 1], f32)
        nc.vector.memset(zero_sb[:], 0.0)
        gq_sb = const.tile([128, KT], f32)
        nc.sync.dma_start(out=gq_sb[:], in_=gq)
        bq_sb = const.tile([128, KT], f32)
        nc.sync.dma_start(out=bq_sb[:], in_=bq)
        gk_sb = const.tile([128, KT], f32)
        nc.sync.dma_start(out=gk_sb[:], in_=gk)
        bk_sb = const.tile([128, KT], f32)
        nc.sync.dma_start(out=bk_sb[:], in_=bk)

        pers = top.enter_context(tc.tile_pool(name="pers", bufs=1))
        qT = pers.tile([128, KT, TQ], bf16)       # LN(q)^T, [E, tok_q]
        kTt = pers.tile([128, KT, TK], bf16)      # LN(k)^T, [E, tok_k]
        vsb = pers.tile([128, TK // 128, H * VW], bf16)  # v + ones col per head
        oT = pers.tile([128, KT, TQ], f32r)        # attention output^T, [E, tok_q]
        nc.vector.memset(
            vsb[:].rearrange("p k (h w) -> p k h w", w=VW)[:, :, :, 64:65], 1.0)

        def project(ps, x_t, w_sb):
            # ps[tok, E] += x^T.T @ W^T  (fp32r, full PE rate)
            for nt in range(2):
                for kt in range(KT):
                    nc.tensor.matmul(
                        ps[:, nt * 512:(nt + 1) * 512],
                        lhsT=x_t[:, kt, :],
                        rhs=w_sb[:, kt, nt * 512:(nt + 1) * 512],
                        start=(kt == 0), stop=(kt == KT - 1))

        def ln_transpose(pools, ps, gam, bet, dstT, mt):
            stats, qn_pool, psT = pools
            st = stats.tile([128, 2, 6], f32, tag="st")
            nc.vector.bn_stats(st[:, 0, :], ps[:, 0:512])
            nc.vector.bn_stats(st[:, 1, :], ps[:, 512:1024])
            mv = stats.tile([128, 2], f32, tag="mv")
            nc.vector.bn_aggr(mv[:], st[:])
            lnv = stats.tile([128, 1], f32, tag="lnv")
            nc.scalar.activation(lnv[:], mv[:, 1:2], Ln, bias=eps_sb[:])
            rstd = stats.tile([128, 1], f32, tag="rstd")
            nc.scalar.activation(rstd[:], lnv[:], Exp, bias=zero_sb[:], scale=-0.5)
            qn = qn_pool.tile([128, E], bf16, tag="qn")
            nc.vector.tensor_scalar(out=qn[:], in0=ps[:], scalar1=mv[:, 0:1],
                                    scalar2=rstd[:], op0=SUB, op1=MULT)
            for kt in range(KT):
                tp = psT.tile([128, 128], bf16, tag="tp")
                nc.tensor.transpose(tp[:], qn[:, kt * 128:(kt + 1) * 128], ident[:])
                nc.vector.tensor_scalar(
                    out=dstT[:, kt, mt * 128:(mt + 1) * 128], in0=tp[:],
                    scalar1=gam[:, kt:kt + 1], scalar2=bet[:, kt:kt + 1],
                    op0=MULT, op1=ADD)

        # ---- Phase A: projections + LN + transposes ----
        with ExitStack() as phA:
            wpool = phA.enter_context(tc.tile_pool(name="wA", bufs=2))
            xpool = phA.enter_context(tc.tile_pool(name="xA", bufs=3))
            psA = phA.enter_context(tc.tile_pool(name="psA", bufs=3, space="PSUM"))
            psT = phA.enter_context(tc.tile_pool(name="psT", bufs=2, space="PSUM"))
            stats = phA.enter_context(tc.tile_pool(name="stats", bufs=3))
            qn_pool = phA.enter_context(tc.tile_pool(name="qn", bufs=4))
            pools = (stats, qn_pool, psT)

            # First query-token tile goes ahead of the weight chunks in the
            # DMA queue so the PE can start ~3us in.  Wq and Wv share a slot:
            # Wq is dead after the Q loop, before the K/V loop needs Wv.
            x_first = xpool.tile([128, KT, 128], f32r, tag="xt")
            nc.sync.dma_start(out=x_first[:], in_=xqT[:, :, 0:128])
            wq_sb = wpool.tile([128, KT, E], f32r, tag="w")
            for kt in range(KT):
                nc.sync.dma_start(out=wq_sb[:, kt, :], in_=wqT[:, kt, :])

            for mt in range(TQ // 128):  # Q projection, 1024 query tokens
                if mt == 0:
                    x_t = x_first
                else:
                    x_t = xpool.tile([128, KT, 128], f32r, tag="xt")
                    nc.sync.dma_start(out=x_t[:],
                                      in_=xqT[:, :, mt * 128:(mt + 1) * 128])
                ps = psA.tile([128, E], f32, tag="psA")
                project(ps, x_t, wq_sb)
                ln_transpose(pools, ps, gq_sb, bq_sb, qT, mt)

            wk_sb = wpool.tile([128, KT, E], f32r, tag="w")
            for kt in range(KT):
                nc.sync.dma_start(out=wk_sb[:, kt, :], in_=wkT[:, kt, :])
            wv_sb = wpool.tile([128, KT, E], f32r, tag="w")
            for kt in range(KT):
                nc.sync.dma_start(out=wv_sb[:, kt, :], in_=wvT[:, kt, :])
            for mt in range(TK // 128):  # K + V projections, 2048 kv tokens
                x_t = xpool.tile([128, KT, 128], f32r, tag="xt")
                nc.sync.dma_start(out=x_t[:], in_=xkvT[:, :, mt * 128:(mt + 1) * 128])
                psk = psA.tile([128, E], f32, tag="psA")
                project(psk, x_t, wk_sb)
                psv = psA.tile([128, E], f32, tag="psA")
                project(psv, x_t, wv_sb)
                ln_transpose(pools, psk, gk_sb, bk_sb, kTt, mt)
                for nt in range(2):
                    nc.vector.tensor_copy(
                        out=vsb[:, mt, :].rearrange("p (h w) -> p h w", w=VW)
                            [:, nt * 8:(nt + 1) * 8, 0:64],
                        in_=psv[:, nt * 512:(nt + 1) * 512]
                            .rearrange("p (h w) -> p h w", w=64))

        # ---- Phase B: attention (transposed scores) + output projection ----
        with ExitStack() as phB:
            wpoolB = phB.enter_context(tc.tile_pool(name="wB", bufs=1))
            wo_sb = wpoolB.tile([128, KT, E], f32r)
            for kt in range(KT):
                nc.sync.dma_start(out=wo_sb[:, kt, :], in_=woT[:, kt, :])

            attn = phB.enter_context(ExitStack())
            sT_ps = attn.enter_context(tc.tile_pool(name="sT", bufs=3, space="PSUM"))
            o_ps = attn.enter_context(tc.tile_pool(name="ops", bufs=1, space="PSUM"))
            ex_pool = attn.enter_context(tc.tile_pool(name="ex", bufs=4))
            oraw_pool = attn.enter_context(tc.tile_pool(name="oraw", bufs=2))
            rows = attn.enter_context(tc.tile_pool(name="rows", bufs=3))
            rep_pool = attn.enter_context(tc.tile_pool(name="rep", bufs=2))
            drb = attn.enter_context(tc.tile_pool(name="drb", bufs=2, space="DRAM"))

            def normalize(h, oraw):
                # 1/sum = Exp(-Ln(sum)); broadcast across the 64 head rows by
                # bouncing the row through DRAM and re-reading it with a
                # 0-stride partition AP (no PE/PSUM involved).
                et, pr = h // 2, (h % 2) * 64
                logr = rows.tile([1, TQ], f32, tag="rows")
                nc.scalar.activation(logr[:], oraw[64:65, :], Ln,
                                     bias=zero_sb[0:1, :])
                recr = rows.tile([1, TQ], f32, tag="rows")
                nc.scalar.activation(recr[:], logr[:], Exp,
                                     bias=zero_sb[0:1, :], scale=-1.0)
                dr = drb.tile([1, TQ], f32, tag="drb")
                nc.sync.dma_start(out=dr[:], in_=recr[:])
                rep = rep_pool.tile([64, TQ], f32, tag="rep")
                nc.sync.dma_start(out=rep[:], in_=dr[:].to_broadcast((64, TQ)))
                nc.vector.tensor_mul(out=oT[pr:pr + 64, et, :],
                                     in0=oraw[0:64, :], in1=rep[:])

            pending = None
            for h in range(H):
                et, pr = h // 2, (h % 2) * 64
                o = o_ps.tile([65, TQ], f32, tag="ops")
                for kt in range(TK // 128):
                    sps = sT_ps.tile([128, TQ], f32, tag="sT")
                    for nt in range(2):
                        nc.tensor.matmul(
                            sps[:, nt * 512:(nt + 1) * 512],
                            lhsT=kTt[pr:pr + 64, et, kt * 128:(kt + 1) * 128],
                            rhs=qT[pr:pr + 64, et, nt * 512:(nt + 1) * 512])
                    ex = ex_pool.tile([128, TQ], bf16, tag="ex")
                    nc.scalar.activation(ex[:], sps[:], Exp, bias=zero_sb[:], scale=0.125)
                    for nt in range(2):
                        nc.tensor.matmul(
                            o[:, nt * 512:(nt + 1) * 512],
                            lhsT=vsb[:, kt, h * VW:(h + 1) * VW],
                            rhs=ex[:, nt * 512:(nt + 1) * 512],
                            start=(kt == 0), stop=(kt == TK // 128 - 1))
                oraw = oraw_pool.tile([65, TQ], f32, tag="oraw")
                nc.vector.tensor_copy(out=oraw[:], in_=o[:])
                # Normalize the *previous* head here: its Ln lands in ACT's
                # FIFO after this head's exps, so ACT never stalls waiting on
                # the PE->DVE round trip that produces the sums row.
                if pending is not None:
                    normalize(*pending)
                pending = (h, oraw)
            normalize(*pending)

            attn.close()
            psY = phB.enter_context(tc.tile_pool(name="psY", bufs=2, space="PSUM"))
            ypool = phB.enter_context(tc.tile_pool(name="yp", bufs=2))
            for mt in range(TQ // 128):
                yps = psY.tile([128, E], f32, tag="psY")
                for nt in range(2):
                    for kt in range(KT):
                        nc.tensor.matmul(
                            yps[:, nt * 512:(nt + 1) * 512],
                            lhsT=oT[:, kt, mt * 128:(mt + 1) * 128],
                            rhs=wo_sb[:, kt, nt * 512:(nt + 1) * 512],
                            start=(kt == 0), stop=(kt == KT - 1))
                ysb = ypool.tile([128, E], f32, tag="ysb")
                nc.any.tensor_copy(out=ysb[:], in_=yps[:])
                nc.sync.dma_start(out=y[mt * 128:(mt + 1) * 128, :], in_=ysb[:])

    nc.compile()
    return nc


def _tile_t(a):
    # [tok, E] -> [128, KT, tok] with [p, kt, t] = a[t, kt*128+p]
    return np.ascontiguousarray(a.T.reshape(KT, 128, -1).transpose(1, 0, 2))


def _shard(inputs):
    wq = _tile_t(np.ascontiguousarray(inputs["Wq"]))   # Wq.T tiled
    wk = _tile_t(np.ascontiguousarray(inputs["Wk"]))
    wv = _tile_t(np.ascontiguousarray(inputs["Wv"]))
    wo = _tile_t(np.ascontiguousarray(inputs["Wo"]))
    gq = np.ascontiguousarray(inputs["q_ln_gamma"].reshape(KT, 128).T)
    bq = np.ascontiguousarray(inputs["q_ln_beta"].reshape(KT, 128).T)
    gk = np.ascontiguousarray(inputs["k_ln_gamma"].reshape(KT, 128).T)
    bk = np.ascontiguousarray(inputs["k_ln_beta"].reshape(KT, 128).T)
    in_maps = []
    for c in range(NCORES):
        b, half = c // 2, c % 2
        xq = inputs["inputs_q"][b, half * TQ:(half + 1) * TQ, :]
        xkv = inputs["inputs_kv"][b]
        in_maps.append({
            "xqT": _tile_t(xq), "xkvT": _tile_t(xkv),
            "wqT": wq, "wkT": wk, "wvT": wv, "woT": wo,
            "gq": gq, "bq": bq, "gk": gk, "bk": bk,
        })
    return in_maps


def run_sharded(inputs, trace=False):
    from concourse.bass_utils import run_bass_kernel_spmd
    if "nc" not in _CACHE:
        _CACHE["nc"] = _build()
    nc = _CACHE["nc"]
    in_maps = _shard(inputs)
    try:
        res = run_bass_kernel_spmd(nc, in_maps, core_ids=list(range(NCORES)),
                                   trace=trace)
    except ModuleNotFoundError:
        # NTFF profiling hooks absent in this container; run untraced.
        res = run_bass_kernel_spmd(nc, in_maps, core_ids=list(range(NCORES)))
    out = np.empty((B, T, E), np.float32)
    for c in range(NCORES):
        b, half = c // 2, c % 2
        out[b, half * TQ:(half + 1) * TQ, :] = res.results[c]["y"]
    return out, res


def kernel(**inputs):
    inputs = {k: np.asarray(v, np.float32) for k, v in inputs.items()}
    out, _ = run_sharded(inputs,
                         trace=bool(int(os.environ.get("KERNEL_TRACE", "0"))))
    return out
